# revision 22
# baseline (speedup 1.0000x reference)
"""Distributed attention kernel for Trainium2 (8 NeuronCores, SPMD).

Problem: B=16 batches of single-query attention over NK=4096 keys,
EMBED=1024, ATTN=256, with a shared kq projection and a v projection.

Math restructuring (exact up to float reassociation):
  - scores = (q@W_kq + b_kq) @ (k@W_kq + b_kq)^T / 16
           = k @ qt + const            where qt = W_kq @ (W_kq^T q + b_kq) / 16
    (the constant offsets every score equally -> softmax invariant, dropped)
  - out = softmax(scores) @ (v@W_v + b_v)
        = (attn @ v) @ W_v + b_v       (attn sums to 1)
This removes the O(NK*E*E) v-projection and O(NK*E*A) k-projection
entirely; the kernel is HBM-bandwidth bound streaming k and v once.

Sharding: data-parallel over batch, 2 batches per core; the small
weights are replicated (W_kq additionally pre-transposed on the host).
Softmax uses unnormalized exp (scores ~ N(0,1), no overflow in fp32)
with 1/Z folded into the output projection.

Raw bass (not Tile): this toolchain's walrus build rejects >1 embedded
sync-wait per compute instruction, which Tile's scheduler emits; raw
bass uses standalone sequencer waits instead.

Engine plan per batch:
  sync : all DMAs (weights once; k/v streamed in 2MB chunks, 3 buffers)
  PE   : qp=W_kq^T q; qt row; qt broadcast (ones outer product);
         Z=sum(exp) partition-reduce; w = attn_unnorm @ v (moving-v);
         w row->col fold (ones outer product); out = (w/Z) @ W_v
  DVE  : qp+b_kq; qt_bcast copy; s=k.qt via fused tensor_tensor_reduce
         (in-place on k tiles); 1/Z; w_col copy; out + b_v
  ACT  : qt_ps->sbuf (x 1/16); exp(s) with row-sum accum; w_ps->sbuf (x 1/Z)

PSUM bank map (PE-W vs DVE/ACT-R hazards serialized via the sem chain):
  bank 0    : qp (cols 0:2), Z (col 4), fold (cols 8:16)
  banks 1-2 : qt row / out row (disjoint lifetimes)
  banks 3-4 : qt broadcast [128, 1024]
  banks 5-6 : w accumulator row
"""

import contextlib

import numpy as np

try:
    import concourse.bass as bass  # noqa: F401
except ImportError:  # fallback if site path isn't preloaded
    import sys

    sys.path.insert(0, "/opt/trn_rl_repo")

B = 16
NCORES = 8
BPC = B // NCORES  # batches per core
NK = 4096
E = 1024
A = 256
NSUB = NK // 128  # 32 token subtiles of 128
CHUNK = 4  # subtiles per DMA chunk (2 MB)
NCHUNK = NSUB // CHUNK
KBUFS = 3
VBUFS = 3


def _build_nc():
    import concourse.bass as bass
    from concourse import mybir

    FP = mybir.dt.float32
    AL = mybir.AluOpType
    AF = mybir.ActivationFunctionType

    nc = bass.Bass()
    q_d = nc.declare_dram_parameter("q", [BPC, E], FP, isOutput=False)
    k_d = nc.declare_dram_parameter("k", [BPC, NK, E], FP, isOutput=False)
    v_d = nc.declare_dram_parameter("v", [BPC, NK, E], FP, isOutput=False)
    wkq_d = nc.declare_dram_parameter("W_kq", [E, A], FP, isOutput=False)
    wkqT_d = nc.declare_dram_parameter("W_kqT", [A, E], FP, isOutput=False)
    bkq_d = nc.declare_dram_parameter("b_kq", [A], FP, isOutput=False)
    wv_d = nc.declare_dram_parameter("W_v", [E, E], FP, isOutput=False)
    bv_d = nc.declare_dram_parameter("b_v", [E], FP, isOutput=False)
    out_d = nc.declare_dram_parameter("out", [BPC, E], FP, isOutput=True)

    with contextlib.ExitStack() as st:
        # ---- SBUF ----
        wkq_sb = st.enter_context(nc.sbuf_tensor([128, 8, A], FP))
        wkqT_sb = st.enter_context(nc.sbuf_tensor([128, 2, E], FP))
        wv_sb = st.enter_context(nc.sbuf_tensor([128, 8, E], FP))
        q_col = st.enter_context(nc.sbuf_tensor([128, BPC * 8], FP))
        bkq_col = st.enter_context(nc.sbuf_tensor([128, 2], FP))
        bv_row = st.enter_context(nc.sbuf_tensor([1, E], FP))
        ones_row = st.enter_context(nc.sbuf_tensor([1, 128], FP))
        ones_col = st.enter_context(nc.sbuf_tensor([128, 1], FP))
        def sb(name, shape):
            return st.enter_context(nc.sbuf_tensor(name, shape, FP))

        kt = [sb(f"kt{i}", [128, CHUNK, E]) for i in range(KBUFS)]
        vt = [sb(f"vt{i}", [128, CHUNK, E]) for i in range(VBUFS)]
        # per-batch smalls (duplicated: no cross-batch WAR analysis needed)
        qp_sb = [sb(f"qp_sb{b}", [128, 2]) for b in range(BPC)]
        qt_sb = [sb(f"qt_sb{b}", [1, E]) for b in range(BPC)]
        qtb_sb = [sb(f"qtb_sb{b}", [128, E]) for b in range(BPC)]
        smat = [sb(f"smat{b}", [128, NSUB]) for b in range(BPC)]
        pmat = [sb(f"pmat{b}", [128, NSUB]) for b in range(BPC)]
        psums = [sb(f"psums{b}", [128, 1]) for b in range(BPC)]
        invz = [sb(f"invz{b}", [1, 1]) for b in range(BPC)]
        w_row = [sb(f"w_row{b}", [1, E]) for b in range(BPC)]
        w_col = [sb(f"w_col{b}", [128, 8]) for b in range(BPC)]
        o_sb = [sb(f"o_sb{b}", [1, E]) for b in range(BPC)]

        # ---- PSUM (static bank map) ----
        ps_small = st.enter_context(nc.psum_tensor([128, 512], FP))   # bank 0
        ps_a = st.enter_context(nc.psum_tensor([128, 1024], FP))      # banks 1-2
        ps_b = st.enter_context(nc.psum_tensor([128, 1024], FP))      # banks 3-4
        ps_w = st.enter_context(nc.psum_tensor([128, 1024], FP))      # banks 5-6

        # ---- semaphores ----
        # DMA queues complete out of order, so one counting sem cannot tell
        # WHICH transfer finished: use one sem per k/v buffer slot (a slot's
        # DMAs are serialized by the flow-control waits) and group sems that
        # are only ever waited at their full-group totals.
        sW = st.enter_context(nc.semaphore("sW"))      # wkq+wkqT+q+bkq -> 64
        sWV = st.enter_context(nc.semaphore("sWV"))    # wv -> 16
        sBV = st.enter_context(nc.semaphore("sBV"))    # bv -> 16
        sK = [st.enter_context(nc.semaphore(f"sK{i}")) for i in range(KBUFS)]
        sV = [st.enter_context(nc.semaphore(f"sV{i}")) for i in range(VBUFS)]
        sOUT = st.enter_context(nc.semaphore("sOUT"))
        sPE = st.enter_context(nc.semaphore("sPE"))
        sDVE = st.enter_context(nc.semaphore("sDVE"))
        sACT = st.enter_context(nc.semaphore("sACT"))

        blk = st.enter_context(nc.Block())

        # ---------- semaphore tick bookkeeping (python-side) ----------
        # PE ticks
        PE_QP = [1, 15]        # after qp MMs of batch b
        PE_QT = [2, 16]
        PE_QTB = [3, 17]
        PE_Z = [4, 18]
        PE_WCHUNK = [[5 + c for c in range(NCHUNK)],
                     [19 + c for c in range(NCHUNK)]]   # after w MMs chunk c
        PE_FOLD = [13, 27]
        PE_PROJ = [14, 28]
        # DVE ticks (2 memset incs first)
        DVE_QPSB = [3, 16]
        DVE_QTBSB = [4, 17]
        DVE_TTR = [[5 + c for c in range(NCHUNK)],
                   [18 + c for c in range(NCHUNK)]]     # after TTRs chunk c
        DVE_INVZ = [13, 26]
        DVE_WCOL = [14, 27]
        DVE_OSB = [15, 28]
        # ACT ticks
        ACT_QTSB = [1, 4]
        ACT_EXP = [2, 5]
        ACT_WROW = [3, 6]

        # ---------- SYNC: all DMAs ----------
        @blk.sync
        def _(sync):
            sync.dma_start(
                out=wkq_sb[:], in_=wkq_d[:].rearrange("(dc p) a -> p dc a", p=128)
            ).then_inc(sW, 16)
            sync.dma_start(
                out=wkqT_sb[:], in_=wkqT_d[:].rearrange("(ac p) d -> p ac d", p=128)
            ).then_inc(sW, 16)
            with nc.allow_non_contiguous_dma(reason="tiny columnar q/bkq loads"):
                sync.dma_start(
                    out=q_col[:], in_=q_d[:].rearrange("b (c p) -> p (b c)", p=128)
                ).then_inc(sW, 16)
                sync.dma_start(
                    out=bkq_col[:], in_=bkq_d[:].rearrange("(c p) -> p c", p=128)
                ).then_inc(sW, 16)
            sync.dma_start(out=bv_row[:], in_=bv_d[:][None, :]).then_inc(sBV, 16)

            # k batch 0 (wv interleaved after k0 so first TTR data arrives early)
            for b in range(BPC):
                k_b = k_d[:][b].rearrange("(s p) d -> p s d", p=128)
                for c in range(NCHUNK):
                    g = b * NCHUNK + c
                    if g >= KBUFS:
                        gp = g - KBUFS
                        sync.wait_ge(sDVE, DVE_TTR[gp // NCHUNK][gp % NCHUNK])
                    sync.dma_start(
                        out=kt[g % KBUFS][:],
                        in_=k_b[:, c * CHUNK:(c + 1) * CHUNK, :],
                    ).then_inc(sK[g % KBUFS], 16)
                if b == 0:
                    sync.dma_start(
                        out=wv_sb[:],
                        in_=wv_d[:].rearrange("(dc p) e -> p dc e", p=128),
                    ).then_inc(sWV, 16)
                v_b = v_d[:][b].rearrange("(s p) d -> p s d", p=128)
                for c in range(NCHUNK):
                    g = b * NCHUNK + c
                    if g >= VBUFS:
                        gp = g - VBUFS
                        sync.wait_ge(sPE, PE_WCHUNK[gp // NCHUNK][gp % NCHUNK])
                    sync.dma_start(
                        out=vt[g % VBUFS][:],
                        in_=v_b[:, c * CHUNK:(c + 1) * CHUNK, :],
                    ).then_inc(sV[g % VBUFS], 16)

            for b in range(BPC):
                sync.wait_ge(sDVE, DVE_OSB[b])
                sync.dma_start(out=out_d[:][b:b + 1, :], in_=o_sb[b][:]).then_inc(
                    sOUT, 16)
            sync.wait_ge(sOUT, BPC * 16)

        # ---------- PE ----------
        @blk.tensor
        def _(tensor):
            tensor.wait_ge(sW, 64)  # wkq + wkqT + q_col + bkq
            for b in range(BPC):
                # qp = W_kq^T q  -> ps_small[:, 0:2]
                for ac in range(2):
                    for dc in range(8):
                        mm = tensor.matmul(
                            out=ps_small[:, ac:ac + 1],
                            lhsT=wkq_sb[:, dc, ac * 128:(ac + 1) * 128],
                            rhs=q_col[:, b * 8 + dc:b * 8 + dc + 1],
                            start=(dc == 0), stop=(dc == 7),
                        )
                mm.then_inc(sPE, 1)

                # qt row = qp^T @ W_kqT  -> ps_a[0:1, :]
                tensor.wait_ge(sDVE, DVE_QPSB[b])
                for ac in range(2):
                    for nh in range(2):
                        mm = tensor.matmul(
                            out=ps_a[0:1, nh * 512:(nh + 1) * 512],
                            lhsT=qp_sb[b][:, ac:ac + 1],
                            rhs=wkqT_sb[:, ac, nh * 512:(nh + 1) * 512],
                            start=(ac == 0), stop=(ac == 1),
                        )
                mm.then_inc(sPE, 1)

                # qt broadcast to 128 partitions -> ps_b
                tensor.wait_ge(sACT, ACT_QTSB[b])
                for nh in range(2):
                    mm = tensor.matmul(
                        out=ps_b[:, nh * 512:(nh + 1) * 512],
                        lhsT=ones_row[:],
                        rhs=qt_sb[b][0:1, nh * 512:(nh + 1) * 512],
                        start=True, stop=True,
                    )
                mm.then_inc(sPE, 1)

                # Z = sum_p(psums) -> ps_small[0:1, 4:5]
                tensor.wait_ge(sACT, ACT_EXP[b])
                tensor.matmul(
                    out=ps_small[0:1, 4:5], lhsT=psums[b][:], rhs=ones_col[:],
                    start=True, stop=True,
                ).then_inc(sPE, 1)

                # w = attn_unnorm @ v -> ps_w[0:1, :]
                for c in range(NCHUNK):
                    g = b * NCHUNK + c
                    tensor.wait_ge(sV[g % VBUFS], (g // VBUFS + 1) * 16)
                    for j in range(CHUNK):
                        t = c * CHUNK + j
                        for nh in range(2):
                            mm = tensor.matmul(
                                out=ps_w[0:1, nh * 512:(nh + 1) * 512],
                                lhsT=pmat[b][:, t:t + 1],
                                rhs=vt[(b * NCHUNK + c) % VBUFS][
                                    :, j, nh * 512:(nh + 1) * 512],
                                start=(t == 0), stop=(t == NSUB - 1),
                            )
                    mm.then_inc(sPE, 1)

                # fold w row -> columns via ones outer product -> ps_small[:, 8:16]
                tensor.wait_ge(sACT, ACT_WROW[b])
                for dc in range(8):
                    mm = tensor.matmul(
                        out=ps_small[:, 8 + dc:9 + dc],
                        lhsT=w_row[b][0:1, dc * 128:(dc + 1) * 128],
                        rhs=ones_row[0:1, 0:1],
                        start=True, stop=True,
                    )
                mm.then_inc(sPE, 1)

                # out row = (w/Z)^T @ W_v -> ps_a[0:1, :]
                tensor.wait_ge(sDVE, DVE_WCOL[b])
                if b == 0:
                    tensor.wait_ge(sWV, 16)  # wv
                for dc in range(8):
                    for nh in range(2):
                        mm = tensor.matmul(
                            out=ps_a[0:1, nh * 512:(nh + 1) * 512],
                            lhsT=w_col[b][:, dc:dc + 1],
                            rhs=wv_sb[:, dc, nh * 512:(nh + 1) * 512],
                            start=(dc == 0), stop=(dc == 7),
                        )
                mm.then_inc(sPE, 1)

        # ---------- DVE ----------
        @blk.vector
        def _(vector):
            vector.memset(ones_row[:], 1.0).then_inc(sDVE, 1)
            vector.memset(ones_col[:], 1.0).then_inc(sDVE, 1)
            for b in range(BPC):
                if b == 0:
                    vector.wait_ge(sW, 64)  # bkq
                vector.wait_ge(sPE, PE_QP[b])
                vector.tensor_add(qp_sb[b][:], ps_small[:, 0:2], bkq_col[:]) \
                    .then_inc(sDVE, 1)

                vector.wait_ge(sPE, PE_QTB[b])
                vector.tensor_copy(out=qtb_sb[b][:], in_=ps_b[:]).then_inc(sDVE, 1)

                # self-wait: force qtb_sb copy completion before TTRs read it
                vector.wait_ge(sDVE, DVE_QTBSB[b])
                for c in range(NCHUNK):
                    g = b * NCHUNK + c
                    vector.wait_ge(sK[g % KBUFS], (g // KBUFS + 1) * 16)
                    buf = kt[g % KBUFS]
                    for j in range(CHUNK):
                        t = c * CHUNK + j
                        # fused dot product: out=(k*1)*qt, accum_out=row-sum
                        ttr = vector.scalar_tensor_tensor(
                            out=buf[:, j, :], in0=buf[:, j, :], scalar=1.0,
                            in1=qtb_sb[b][:],
                            op0=AL.mult, op1=AL.mult,
                            accum_out=smat[b][:, t:t + 1],
                        )
                    ttr.then_inc(sDVE, 1)

                vector.wait_ge(sPE, PE_Z[b])
                vector.reciprocal(invz[b][:], ps_small[0:1, 4:5]).then_inc(sDVE, 1)

                vector.wait_ge(sPE, PE_FOLD[b])
                vector.tensor_copy(out=w_col[b][:], in_=ps_small[:, 8:16]) \
                    .then_inc(sDVE, 1)

                vector.wait_ge(sPE, PE_PROJ[b])
                if b == 0:
                    vector.wait_ge(sBV, 16)
                vector.tensor_add(o_sb[b][:], ps_a[0:1, :], bv_row[:]) \
                    .then_inc(sDVE, 1)

        # ---------- ACT (scalar) ----------
        @blk.scalar
        def _(scalar):
            for b in range(BPC):
                scalar.wait_ge(sPE, PE_QT[b])
                scalar.mul(qt_sb[b][:], ps_a[0:1, :], 1.0 / 16.0).then_inc(sACT, 1)

                scalar.wait_ge(sDVE, DVE_TTR[b][NCHUNK - 1])
                scalar.activation(
                    out=pmat[b][:], in_=smat[b][:], func=AF.Exp,
                    accum_out=psums[b][:],
                ).then_inc(sACT, 1)

                scalar.wait_ge(sPE, PE_WCHUNK[b][NCHUNK - 1])
                scalar.wait_ge(sDVE, DVE_INVZ[b])
                scalar.activation(
                    out=w_row[b][:], in_=ps_w[0:1, :], func=AF.Copy,
                    bias=0.0, scale=invz[b][0:1, 0:1],
                ).then_inc(sACT, 1)

    return nc


_NC_CACHE = None


def get_nc():
    global _NC_CACHE
    if _NC_CACHE is None:
        _NC_CACHE = _build_nc()
    return _NC_CACHE


def make_in_maps(q, k, v, W_kq, b_kq, W_v, b_v):
    """Shard full inputs over 8 cores: batch-parallel, weights replicated."""
    q = np.ascontiguousarray(np.asarray(q, dtype=np.float32).reshape(B, E))
    k = np.ascontiguousarray(np.asarray(k, dtype=np.float32))
    v = np.ascontiguousarray(np.asarray(v, dtype=np.float32))
    W_kq = np.ascontiguousarray(np.asarray(W_kq, dtype=np.float32))
    W_kqT = np.ascontiguousarray(W_kq.T)
    b_kq = np.ascontiguousarray(np.asarray(b_kq, dtype=np.float32))
    W_v = np.ascontiguousarray(np.asarray(W_v, dtype=np.float32))
    b_v = np.ascontiguousarray(np.asarray(b_v, dtype=np.float32))
    in_maps = []
    for i in range(NCORES):
        lo, hi = i * BPC, (i + 1) * BPC
        in_maps.append({
            "q": q[lo:hi],
            "k": k[lo:hi],
            "v": v[lo:hi],
            "W_kq": W_kq,
            "W_kqT": W_kqT,
            "b_kq": b_kq,
            "W_v": W_v,
            "b_v": b_v,
        })
    return in_maps


def kernel(q, k, v, W_kq, b_kq, W_v, b_v):
    from concourse.bass_utils import run_bass_kernel_spmd

    nc = get_nc()
    in_maps = make_in_maps(q, k, v, W_kq, b_kq, W_v, b_v)
    res = run_bass_kernel_spmd(nc, in_maps, core_ids=list(range(NCORES)))
    out = np.concatenate([res.results[i]["out"] for i in range(NCORES)], axis=0)
    return np.ascontiguousarray(out.astype(np.float32))


# revision 24
# speedup vs baseline: 1.5933x; 1.5933x over previous
"""Distributed attention kernel for Trainium2 (8 NeuronCores, SPMD).

Problem: B=16 batches of single-query attention over NK=4096 keys,
EMBED=1024, ATTN=256, with a shared kq projection and a v projection.

Math restructuring (exact up to float reassociation):
  - scores = (q@W_kq + b_kq) @ (k@W_kq + b_kq)^T / 16
           = k @ qt + const            where qt = W_kq @ (W_kq^T q + b_kq) / 16
    (the constant offsets every score equally -> softmax invariant, dropped)
  - out = softmax(scores) @ (v@W_v + b_v)
        = (attn @ v) @ W_v + b_v       (attn sums to 1)
This removes the O(NK*E*E) v-projection and O(NK*E*A) k-projection
entirely; the kernel is HBM-bandwidth bound streaming k and v once.

Sharding: data-parallel over batch, 2 batches per core; the small
weights are replicated (W_kq additionally pre-transposed on the host).
Softmax uses unnormalized exp (scores ~ N(0,1), no overflow in fp32)
with 1/Z folded into the output projection.

Raw bass (not Tile): this toolchain's walrus build rejects >1 embedded
sync-wait per compute instruction, which Tile's scheduler emits; raw
bass uses standalone sequencer waits instead.

Engine plan per batch:
  sync : all DMAs (weights once; k/v streamed in 2MB chunks, 3 buffers)
  PE   : qp=W_kq^T q; qt row; qt broadcast (ones outer product);
         Z=sum(exp) partition-reduce; w = attn_unnorm @ v (moving-v);
         w row->col fold (ones outer product); out = (w/Z) @ W_v
  DVE  : qp+b_kq; qt_bcast copy; s=k.qt via fused tensor_tensor_reduce
         (in-place on k tiles); 1/Z; w_col copy; out + b_v
  ACT  : qt_ps->sbuf (x 1/16); exp(s) with row-sum accum; w_ps->sbuf (x 1/Z)

PSUM bank map (PE-W vs DVE/ACT-R hazards serialized via the sem chain):
  bank 0    : qp (cols 0:2), Z (col 4), fold (cols 8:16)
  banks 1-2 : qt row / out row (disjoint lifetimes)
  banks 3-4 : qt broadcast [128, 1024]
  banks 5-6 : w accumulator row
"""

import contextlib

import numpy as np

try:
    import concourse.bass as bass  # noqa: F401
except ImportError:  # fallback if site path isn't preloaded
    import sys

    sys.path.insert(0, "/opt/trn_rl_repo")

B = 16
NCORES = 8
BPC = B // NCORES  # batches per core
NK = 4096
E = 1024
A = 256
NSUB = NK // 128  # 32 token subtiles of 128
CHUNK = 8  # subtiles per DMA chunk (2 MB in bf16)
NCHUNK = NSUB // CHUNK
KBUFS = 3
VBUFS = 3


def _build_nc():
    import concourse.bass as bass
    from concourse import mybir

    FP = mybir.dt.float32
    BF = mybir.dt.bfloat16
    AL = mybir.AluOpType
    AF = mybir.ActivationFunctionType

    nc = bass.Bass()
    q_d = nc.declare_dram_parameter("q", [BPC, E], FP, isOutput=False)
    k_d = nc.declare_dram_parameter("k", [BPC, NK, E], BF, isOutput=False)
    v_d = nc.declare_dram_parameter("v", [BPC, NK, E], BF, isOutput=False)
    wkq_d = nc.declare_dram_parameter("W_kq", [E, A], FP, isOutput=False)
    wkqT_d = nc.declare_dram_parameter("W_kqT", [A, E], FP, isOutput=False)
    bkq_d = nc.declare_dram_parameter("b_kq", [A], FP, isOutput=False)
    wv_d = nc.declare_dram_parameter("W_v", [E, E], FP, isOutput=False)
    bv_d = nc.declare_dram_parameter("b_v", [E], FP, isOutput=False)
    out_d = nc.declare_dram_parameter("out", [BPC, E], FP, isOutput=True)

    with contextlib.ExitStack() as st:
        # ---- SBUF ----
        wkq_sb = st.enter_context(nc.sbuf_tensor([128, 8, A], FP))
        wkqT_sb = st.enter_context(nc.sbuf_tensor([128, 2, E], FP))
        wv_sb = st.enter_context(nc.sbuf_tensor([128, 8, E], FP))
        q_col = st.enter_context(nc.sbuf_tensor([128, BPC * 8], FP))
        bkq_col = st.enter_context(nc.sbuf_tensor([128, 2], FP))
        bv_row = st.enter_context(nc.sbuf_tensor([1, E], FP))
        ones_row = st.enter_context(nc.sbuf_tensor([1, 128], FP))
        ones_col = st.enter_context(nc.sbuf_tensor([128, 1], FP))
        def sb(name, shape):
            return st.enter_context(nc.sbuf_tensor(name, shape, FP))

        kt = [st.enter_context(nc.sbuf_tensor(f"kt{i}", [128, CHUNK, E], BF))
              for i in range(KBUFS)]
        vt = [st.enter_context(nc.sbuf_tensor(f"vt{i}", [128, CHUNK, E], BF))
              for i in range(VBUFS)]
        # per-batch smalls (duplicated: no cross-batch WAR analysis needed)
        qp_sb = [sb(f"qp_sb{b}", [128, 2]) for b in range(BPC)]
        qt_sb = [sb(f"qt_sb{b}", [1, E]) for b in range(BPC)]
        qtb_sb = [st.enter_context(nc.sbuf_tensor(f"qtb_sb{b}", [128, E], BF))
                  for b in range(BPC)]
        smat = [sb(f"smat{b}", [128, NSUB]) for b in range(BPC)]
        pmat = [st.enter_context(nc.sbuf_tensor(f"pmat{b}", [128, NSUB], BF))
                for b in range(BPC)]
        psums = [sb(f"psums{b}", [128, 1]) for b in range(BPC)]
        invz = [sb(f"invz{b}", [1, 1]) for b in range(BPC)]
        w_row = [sb(f"w_row{b}", [1, E]) for b in range(BPC)]
        w_col = [sb(f"w_col{b}", [128, 8]) for b in range(BPC)]
        o_sb = [sb(f"o_sb{b}", [1, E]) for b in range(BPC)]

        # ---- PSUM (static bank map) ----
        ps_small = st.enter_context(nc.psum_tensor([128, 512], FP))   # bank 0
        ps_a = st.enter_context(nc.psum_tensor([128, 1024], FP))      # banks 1-2
        ps_b = st.enter_context(nc.psum_tensor([128, 1024], FP))      # banks 3-4
        ps_w = st.enter_context(nc.psum_tensor([128, 1024], FP))      # banks 5-6

        # ---- semaphores ----
        # DMA queues complete out of order, so one counting sem cannot tell
        # WHICH transfer finished: use one sem per k/v buffer slot (a slot's
        # DMAs are serialized by the flow-control waits) and group sems that
        # are only ever waited at their full-group totals.
        sW = st.enter_context(nc.semaphore("sW"))      # wkq+wkqT+q+bkq -> 64
        sWV = st.enter_context(nc.semaphore("sWV"))    # wv -> 16
        sBV = st.enter_context(nc.semaphore("sBV"))    # bv -> 16
        sK = [st.enter_context(nc.semaphore(f"sK{i}")) for i in range(KBUFS)]
        sV = [st.enter_context(nc.semaphore(f"sV{i}")) for i in range(VBUFS)]
        sOUT = st.enter_context(nc.semaphore("sOUT"))
        sPE = st.enter_context(nc.semaphore("sPE"))
        sDVE = st.enter_context(nc.semaphore("sDVE"))
        sACT = st.enter_context(nc.semaphore("sACT"))

        blk = st.enter_context(nc.Block())

        # ---------- semaphore tick bookkeeping (python-side) ----------
        PEB = 6 + NCHUNK  # PE incs per batch
        PE_QP = [1 + b * PEB for b in range(BPC)]
        PE_QT = [2 + b * PEB for b in range(BPC)]
        PE_QTB = [3 + b * PEB for b in range(BPC)]
        PE_Z = [4 + b * PEB for b in range(BPC)]
        PE_WCHUNK = [[5 + c + b * PEB for c in range(NCHUNK)] for b in range(BPC)]
        PE_FOLD = [5 + NCHUNK + b * PEB for b in range(BPC)]
        PE_PROJ = [6 + NCHUNK + b * PEB for b in range(BPC)]
        DVEB = 5 + NCHUNK  # DVE incs per batch (after 2 memset incs)
        DVE_QPSB = [3 + b * DVEB for b in range(BPC)]
        DVE_QTBSB = [4 + b * DVEB for b in range(BPC)]
        DVE_TTR = [[5 + c + b * DVEB for c in range(NCHUNK)] for b in range(BPC)]
        DVE_INVZ = [5 + NCHUNK + b * DVEB for b in range(BPC)]
        DVE_WCOL = [6 + NCHUNK + b * DVEB for b in range(BPC)]
        DVE_OSB = [7 + NCHUNK + b * DVEB for b in range(BPC)]
        # ACT ticks
        ACT_QTSB = [1, 4]
        ACT_EXP = [2, 5]
        ACT_WROW = [3, 6]

        # ---------- SYNC: all DMAs ----------
        @blk.sync
        def _(sync):
            sync.dma_start(
                out=wkq_sb[:], in_=wkq_d[:].rearrange("(dc p) a -> p dc a", p=128)
            ).then_inc(sW, 16)
            sync.dma_start(
                out=wkqT_sb[:], in_=wkqT_d[:].rearrange("(ac p) d -> p ac d", p=128)
            ).then_inc(sW, 16)
            with nc.allow_non_contiguous_dma(reason="tiny columnar q/bkq loads"):
                sync.dma_start(
                    out=q_col[:], in_=q_d[:].rearrange("b (c p) -> p (b c)", p=128)
                ).then_inc(sW, 16)
                sync.dma_start(
                    out=bkq_col[:], in_=bkq_d[:].rearrange("(c p) -> p c", p=128)
                ).then_inc(sW, 16)
            sync.dma_start(out=bv_row[:], in_=bv_d[:][None, :]).then_inc(sBV, 16)

            # k batch 0 (wv interleaved after k0 so first TTR data arrives early)
            for b in range(BPC):
                k_b = k_d[:][b].rearrange("(s p) d -> p s d", p=128)
                for c in range(NCHUNK):
                    g = b * NCHUNK + c
                    if g >= KBUFS:
                        gp = g - KBUFS
                        sync.wait_ge(sDVE, DVE_TTR[gp // NCHUNK][gp % NCHUNK])
                    sync.dma_start(
                        out=kt[g % KBUFS][:],
                        in_=k_b[:, c * CHUNK:(c + 1) * CHUNK, :],
                    ).then_inc(sK[g % KBUFS], 16)
                if b == 0:
                    sync.dma_start(
                        out=wv_sb[:],
                        in_=wv_d[:].rearrange("(dc p) e -> p dc e", p=128),
                    ).then_inc(sWV, 16)
                v_b = v_d[:][b].rearrange("(s p) d -> p s d", p=128)
                for c in range(NCHUNK):
                    g = b * NCHUNK + c
                    if g >= VBUFS:
                        gp = g - VBUFS
                        sync.wait_ge(sPE, PE_WCHUNK[gp // NCHUNK][gp % NCHUNK])
                    sync.dma_start(
                        out=vt[g % VBUFS][:],
                        in_=v_b[:, c * CHUNK:(c + 1) * CHUNK, :],
                    ).then_inc(sV[g % VBUFS], 16)

            for b in range(BPC):
                sync.wait_ge(sDVE, DVE_OSB[b])
                sync.dma_start(out=out_d[:][b:b + 1, :], in_=o_sb[b][:]).then_inc(
                    sOUT, 16)
            sync.wait_ge(sOUT, BPC * 16)

        # ---------- PE ----------
        @blk.tensor
        def _(tensor):
            def r(ap):
                return ap

            tensor.wait_ge(sW, 64)  # wkq + wkqT + q_col + bkq
            for b in range(BPC):
                # qp = W_kq^T q  -> ps_small[:, 0:2]
                for ac in range(2):
                    for dc in range(8):
                        mm = tensor.matmul(
                            out=ps_small[:, ac:ac + 1],
                            lhsT=r(wkq_sb[:, dc, ac * 128:(ac + 1) * 128]),
                            rhs=r(q_col[:, b * 8 + dc:b * 8 + dc + 1]),
                            start=(dc == 0), stop=(dc == 7),
                        )
                mm.then_inc(sPE, 1)

                # qt row = qp^T @ W_kqT  -> ps_a[0:1, :]
                tensor.wait_ge(sDVE, DVE_QPSB[b])
                for ac in range(2):
                    for nh in range(2):
                        mm = tensor.matmul(
                            out=ps_a[0:1, nh * 512:(nh + 1) * 512],
                            lhsT=r(qp_sb[b][:, ac:ac + 1]),
                            rhs=r(wkqT_sb[:, ac, nh * 512:(nh + 1) * 512]),
                            start=(ac == 0), stop=(ac == 1),
                        )
                mm.then_inc(sPE, 1)

                # qt broadcast to 128 partitions -> ps_b
                tensor.wait_ge(sACT, ACT_QTSB[b])
                for nh in range(2):
                    mm = tensor.matmul(
                        out=ps_b[:, nh * 512:(nh + 1) * 512],
                        lhsT=r(ones_row[:]),
                        rhs=r(qt_sb[b][0:1, nh * 512:(nh + 1) * 512]),
                        start=True, stop=True,
                    )
                mm.then_inc(sPE, 1)

                # Z = sum_p(psums) -> ps_small[0:1, 4:5]
                tensor.wait_ge(sACT, ACT_EXP[b])
                tensor.matmul(
                    out=ps_small[0:1, 4:5], lhsT=r(psums[b][:]), rhs=r(ones_col[:]),
                    start=True, stop=True,
                ).then_inc(sPE, 1)

                # w = attn_unnorm @ v -> ps_w[0:1, :]
                for c in range(NCHUNK):
                    g = b * NCHUNK + c
                    tensor.wait_ge(sV[g % VBUFS], (g // VBUFS + 1) * 16)
                    for j in range(CHUNK):
                        t = c * CHUNK + j
                        for nh in range(2):
                            mm = tensor.matmul(
                                out=ps_w[0:1, nh * 512:(nh + 1) * 512],
                                lhsT=r(pmat[b][:, t:t + 1]),
                                rhs=r(vt[(b * NCHUNK + c) % VBUFS][
                                    :, j, nh * 512:(nh + 1) * 512]),
                                start=(t == 0), stop=(t == NSUB - 1),
                            )
                    mm.then_inc(sPE, 1)

                # fold w row -> columns via ones outer product -> ps_small[:, 8:16]
                tensor.wait_ge(sACT, ACT_WROW[b])
                for dc in range(8):
                    mm = tensor.matmul(
                        out=ps_small[:, 8 + dc:9 + dc],
                        lhsT=r(w_row[b][0:1, dc * 128:(dc + 1) * 128]),
                        rhs=r(ones_row[0:1, 0:1]),
                        start=True, stop=True,
                    )
                mm.then_inc(sPE, 1)

                # out row = (w/Z)^T @ W_v -> ps_a[0:1, :]
                tensor.wait_ge(sDVE, DVE_WCOL[b])
                if b == 0:
                    tensor.wait_ge(sWV, 16)  # wv
                for dc in range(8):
                    for nh in range(2):
                        mm = tensor.matmul(
                            out=ps_a[0:1, nh * 512:(nh + 1) * 512],
                            lhsT=r(w_col[b][:, dc:dc + 1]),
                            rhs=r(wv_sb[:, dc, nh * 512:(nh + 1) * 512]),
                            start=(dc == 0), stop=(dc == 7),
                        )
                mm.then_inc(sPE, 1)

        # ---------- DVE ----------
        @blk.vector
        def _(vector):
            vector.memset(ones_row[:], 1.0).then_inc(sDVE, 1)
            vector.memset(ones_col[:], 1.0).then_inc(sDVE, 1)
            for b in range(BPC):
                if b == 0:
                    vector.wait_ge(sW, 64)  # bkq
                vector.wait_ge(sPE, PE_QP[b])
                vector.tensor_add(qp_sb[b][:], ps_small[:, 0:2], bkq_col[:]) \
                    .then_inc(sDVE, 1)

                vector.wait_ge(sPE, PE_QTB[b])
                vector.tensor_copy(out=qtb_sb[b][:], in_=ps_b[:]).then_inc(sDVE, 1)

                # self-wait: force qtb_sb copy completion before TTRs read it
                vector.wait_ge(sDVE, DVE_QTBSB[b])
                for c in range(NCHUNK):
                    g = b * NCHUNK + c
                    vector.wait_ge(sK[g % KBUFS], (g // KBUFS + 1) * 16)
                    buf = kt[g % KBUFS]
                    for j in range(CHUNK):
                        t = c * CHUNK + j
                        # fused dot product: out=(k*1)*qt, accum_out=row-sum
                        ttr = vector.scalar_tensor_tensor(
                            out=buf[:, j, :], in0=buf[:, j, :], scalar=1.0,
                            in1=qtb_sb[b][:],
                            op0=AL.mult, op1=AL.mult,
                            accum_out=smat[b][:, t:t + 1],
                        )
                    ttr.then_inc(sDVE, 1)

                vector.wait_ge(sPE, PE_Z[b])
                vector.reciprocal(invz[b][:], ps_small[0:1, 4:5]).then_inc(sDVE, 1)

                vector.wait_ge(sPE, PE_FOLD[b])
                vector.tensor_copy(out=w_col[b][:], in_=ps_small[:, 8:16]) \
                    .then_inc(sDVE, 1)

                vector.wait_ge(sPE, PE_PROJ[b])
                if b == 0:
                    vector.wait_ge(sBV, 16)
                vector.tensor_add(o_sb[b][:], ps_a[0:1, :], bv_row[:]) \
                    .then_inc(sDVE, 1)

        # ---------- ACT (scalar) ----------
        @blk.scalar
        def _(scalar):
            for b in range(BPC):
                scalar.wait_ge(sPE, PE_QT[b])
                scalar.mul(qt_sb[b][:], ps_a[0:1, :], 1.0 / 16.0).then_inc(sACT, 1)

                scalar.wait_ge(sDVE, DVE_TTR[b][NCHUNK - 1])
                scalar.activation(
                    out=pmat[b][:], in_=smat[b][:], func=AF.Exp,
                    accum_out=psums[b][:],
                ).then_inc(sACT, 1)

                scalar.wait_ge(sPE, PE_WCHUNK[b][NCHUNK - 1])
                scalar.wait_ge(sDVE, DVE_INVZ[b])
                scalar.activation(
                    out=w_row[b][:], in_=ps_w[0:1, :], func=AF.Copy,
                    bias=0.0, scale=invz[b][0:1, 0:1],
                ).then_inc(sACT, 1)

    return nc


_NC_CACHE = None


def get_nc():
    global _NC_CACHE
    if _NC_CACHE is None:
        _NC_CACHE = _build_nc()
    return _NC_CACHE


def make_in_maps(q, k, v, W_kq, b_kq, W_v, b_v):
    """Shard full inputs over 8 cores: batch-parallel, weights replicated.
    k and v are cast to bfloat16 on the host (compute dtype of the two
    streaming contractions; halves HBM traffic)."""
    import ml_dtypes

    bf16 = ml_dtypes.bfloat16
    q = np.ascontiguousarray(np.asarray(q, dtype=np.float32).reshape(B, E))
    k = np.ascontiguousarray(np.asarray(k, dtype=np.float32).astype(bf16))
    v = np.ascontiguousarray(np.asarray(v, dtype=np.float32).astype(bf16))
    W_kq = np.ascontiguousarray(np.asarray(W_kq, dtype=np.float32))
    W_kqT = np.ascontiguousarray(W_kq.T)
    b_kq = np.ascontiguousarray(np.asarray(b_kq, dtype=np.float32))
    W_v = np.ascontiguousarray(np.asarray(W_v, dtype=np.float32))
    b_v = np.ascontiguousarray(np.asarray(b_v, dtype=np.float32))
    in_maps = []
    for i in range(NCORES):
        lo, hi = i * BPC, (i + 1) * BPC
        in_maps.append({
            "q": q[lo:hi],
            "k": k[lo:hi],
            "v": v[lo:hi],
            "W_kq": W_kq,
            "W_kqT": W_kqT,
            "b_kq": b_kq,
            "W_v": W_v,
            "b_v": b_v,
        })
    return in_maps


def kernel(q, k, v, W_kq, b_kq, W_v, b_v):
    from concourse.bass_utils import run_bass_kernel_spmd

    nc = get_nc()
    in_maps = make_in_maps(q, k, v, W_kq, b_kq, W_v, b_v)
    res = run_bass_kernel_spmd(nc, in_maps, core_ids=list(range(NCORES)))
    out = np.concatenate([res.results[i]["out"] for i in range(NCORES)], axis=0)
    return np.ascontiguousarray(out.astype(np.float32))


# revision 26
# speedup vs baseline: 1.9112x; 1.1996x over previous
"""Distributed attention kernel for Trainium2 (8 NeuronCores, SPMD).

Problem: B=16 batches of single-query attention over NK=4096 keys,
EMBED=1024, ATTN=256, with a shared kq projection and a v projection.

Math restructuring (exact up to float reassociation):
  - scores = (q@W_kq + b_kq) @ (k@W_kq + b_kq)^T / 16
           = k @ qt + const            where qt = W_kq @ (W_kq^T q + b_kq) / 16
    (the constant offsets every score equally -> softmax invariant, dropped)
  - out = softmax(scores) @ (v@W_v + b_v)
        = (attn @ v) @ W_v + b_v       (attn sums to 1)
This removes the O(NK*E*E) v-projection and O(NK*E*A) k-projection
entirely; the kernel is HBM-bandwidth bound streaming k and v once.

Sharding: data-parallel over batch, 2 batches per core. k, v, W_v are
cast to bf16 on the host (compute dtype of the streaming contractions,
halves HBM traffic); W_kq is additionally passed pre-transposed.

Token layout is p-major ("(p s) d"): partition p holds NSUB consecutive
token rows, so each chunk DMA is one 16KB-contiguous run per partition
(128 descriptors, not 1024). k and v use the same permutation, and
softmax is globally permutation-invariant, so results are unchanged.

s = k.qt is split across two engines per 128-token tile:
  DVE : prod = k_tile * qt_bcast   (bf16 tensor_tensor, 2x mode)
  ACT : activation(Copy) with accum_out -> row-sum = scores column
Softmax uses unnormalized exp (scores ~ N(0,1), no overflow in fp32)
with 1/Z folded into the w_row evacuation.

Raw bass (not Tile): this toolchain's walrus build rejects >1 embedded
sync-wait per compute instruction, which Tile's scheduler emits; raw
bass uses standalone sequencer waits, with explicit semaphore ticks
precomputed in python (the *_seq tables below).

PSUM bank map (PE-W vs DVE/ACT-R hazards serialized via the sem chain):
  bank 0    : qp_row [0:1,256:512], qp_col [:,0:2], Z [0:1,4:5], fold [:,8:16]
  banks 1-2 : qt row / out row (disjoint lifetimes)
  banks 3-4 : qt broadcast [128, 1024]
  banks 5-6 : w accumulator row
"""

import contextlib

import numpy as np

try:
    import concourse.bass as bass  # noqa: F401
except ImportError:  # fallback if site path isn't preloaded
    import sys

    sys.path.insert(0, "/opt/trn_rl_repo")

B = 16
NCORES = 8
BPC = B // NCORES  # batches per core
NK = 4096
E = 1024
A = 256
NSUB = NK // 128   # 32 token subtiles of 128
CHUNK = 8          # subtiles per DMA chunk (2 MB in bf16)
NCHUNK = NSUB // CHUNK
KBUFS = 3
VBUFS = 3
NSCR = CHUNK       # prod scratch depth = one chunk, so a chunk's multiplies
                   # only wait on the PREVIOUS chunk's reduces (no cycle with
                   # the small-chain ops interleaved into the ACT stream)


def _build_nc():
    import concourse.bass as bass
    from concourse import mybir

    FP = mybir.dt.float32
    BF = mybir.dt.bfloat16
    AF = mybir.ActivationFunctionType

    nc = bass.Bass()
    q_d = nc.declare_dram_parameter("q", [BPC, E], FP, isOutput=False)
    k_d = nc.declare_dram_parameter("k", [BPC, NK, E], BF, isOutput=False)
    v_d = nc.declare_dram_parameter("v", [BPC, NK, E], BF, isOutput=False)
    wkq_d = nc.declare_dram_parameter("W_kq", [E, A], FP, isOutput=False)
    wkqT_d = nc.declare_dram_parameter("W_kqT", [A, E], FP, isOutput=False)
    bkq_d = nc.declare_dram_parameter("b_kq", [A], FP, isOutput=False)
    wv_d = nc.declare_dram_parameter("W_v", [E, E], BF, isOutput=False)
    bv_d = nc.declare_dram_parameter("b_v", [E], FP, isOutput=False)
    out_d = nc.declare_dram_parameter("out", [BPC, E], FP, isOutput=True)

    with contextlib.ExitStack() as st:
        def sb(name, shape, dt=FP):
            return st.enter_context(nc.sbuf_tensor(name, shape, dt))

        # ---- SBUF ----
        wkq_sb = sb("wkq_sb", [128, 8, A])
        wkqT_sb = sb("wkqT_sb", [128, 2, E])
        wv_sb = sb("wv_sb", [128, 8, E], BF)
        q_col = sb("q_col", [128, BPC * 8])
        bkq_row = sb("bkq_row", [1, A])
        bv_row = sb("bv_row", [1, E])
        ones_row = sb("ones_row", [1, 128])        # fp32 (qt bcast / qp fold)
        ones_col = sb("ones_col", [128, 1])        # fp32 (Z rhs)
        ones_bf = sb("ones_bf", [1, 128], BF)      # bf16 (w fold rhs)
        kt = [sb(f"kt{i}", [128, CHUNK, E], BF) for i in range(KBUFS)]
        vt = [sb(f"vt{i}", [128, CHUNK, E], BF) for i in range(VBUFS)]
        scr = [sb(f"scr{i}", [128, E], BF) for i in range(NSCR)]
        # per-batch smalls
        qpr_sb = [sb(f"qpr_sb{b}", [1, A]) for b in range(BPC)]
        qp_sb = [sb(f"qp_sb{b}", [128, 2]) for b in range(BPC)]
        qt_sb = [sb(f"qt_sb{b}", [1, E]) for b in range(BPC)]
        qtb_sb = [sb(f"qtb_sb{b}", [128, E], BF) for b in range(BPC)]
        smat = [sb(f"smat{b}", [128, NSUB]) for b in range(BPC)]
        pmat = [sb(f"pmat{b}", [128, NSUB], BF) for b in range(BPC)]
        psums = [sb(f"psums{b}", [128, 1]) for b in range(BPC)]
        invz = [sb(f"invz{b}", [1, 1]) for b in range(BPC)]
        w_row = [sb(f"w_row{b}", [1, E], BF) for b in range(BPC)]
        w_col = [sb(f"w_col{b}", [128, 8], BF) for b in range(BPC)]
        o_sb = [sb(f"o_sb{b}", [1, E]) for b in range(BPC)]

        # ---- PSUM (static bank map) ----
        ps_small = st.enter_context(nc.psum_tensor([128, 512], FP))   # bank 0
        ps_a = st.enter_context(nc.psum_tensor([128, 1024], FP))      # banks 1-2
        ps_b = st.enter_context(nc.psum_tensor([128, 1024], FP))      # banks 3-4
        ps_w = st.enter_context(nc.psum_tensor([128, 1024], FP))      # banks 5-6

        # ---- semaphores ----
        sW = st.enter_context(nc.semaphore("sW"))      # wkq+wkqT+q+bkq -> 64
        sWV = st.enter_context(nc.semaphore("sWV"))    # wv -> 16
        sBV = st.enter_context(nc.semaphore("sBV"))    # bv -> 16
        sK = [st.enter_context(nc.semaphore(f"sK{i}")) for i in range(KBUFS)]
        sV = [st.enter_context(nc.semaphore(f"sV{i}")) for i in range(VBUFS)]
        sOUT = st.enter_context(nc.semaphore("sOUT"))
        sPE = st.enter_context(nc.semaphore("sPE"))
        sDVE = st.enter_context(nc.semaphore("sDVE"))
        sACT = st.enter_context(nc.semaphore("sACT"))

        blk = st.enter_context(nc.Block())

        # ---------- event tick registry ----------
        def ticks(seq):
            return {ev: i + 1 for i, ev in enumerate(seq)}

        pe_seq = []
        for b in range(BPC):
            pe_seq += [f"QPROW{b}", f"QPF{b}", f"QT{b}", f"QTB{b}"]
        for b in range(BPC):
            pe_seq += [f"Z{b}"]
            pe_seq += [f"W{b}C{c}" for c in range(NCHUNK)]
            pe_seq += [f"FOLD{b}", f"PROJ{b}"]
        PE = ticks(pe_seq)

        def mult_ev(b, c, j):
            return f"MUL{b}_{c}_{j}"

        def red_ev(b, c, j):
            return f"RED{b}_{c}_{j}"

        dve_seq = ["MS1", "MS2", "MS3", "QPRSB0", "QPSB0", "QTBSB0"]
        dve_seq += [mult_ev(0, 0, j) for j in range(CHUNK)]
        dve_seq += ["QPRSB1"]
        dve_seq += [mult_ev(0, 1, j) for j in range(CHUNK)]
        dve_seq += ["QPSB1"]
        dve_seq += [mult_ev(0, 2, j) for j in range(CHUNK)]
        dve_seq += ["QTBSB1"]
        dve_seq += [mult_ev(0, 3, j) for j in range(CHUNK)]
        dve_seq += ["INVZ0"]
        dve_seq += [mult_ev(1, 0, j) for j in range(CHUNK)]
        dve_seq += [mult_ev(1, 1, j) for j in range(CHUNK)]
        dve_seq += ["WCOL0", "OSB0"]
        dve_seq += [mult_ev(1, 2, j) for j in range(CHUNK)]
        dve_seq += [mult_ev(1, 3, j) for j in range(CHUNK)]
        dve_seq += ["INVZ1", "WCOL1", "OSB1"]
        DVE = ticks(dve_seq)

        act_seq = ["QTSB0"]
        act_seq += [red_ev(0, 0, j) for j in range(CHUNK)]
        act_seq += ["QTSB1"]
        for c in range(1, NCHUNK):
            act_seq += [red_ev(0, c, j) for j in range(CHUNK)]
        act_seq += ["EXP0"]
        act_seq += [red_ev(1, 0, j) for j in range(CHUNK)]
        act_seq += [red_ev(1, 1, j) for j in range(CHUNK)]
        act_seq += ["WROW0"]
        act_seq += [red_ev(1, 2, j) for j in range(CHUNK)]
        act_seq += [red_ev(1, 3, j) for j in range(CHUNK)]
        act_seq += ["EXP1", "WROW1"]
        ACT = ticks(act_seq)

        def tile_red_tick(T):
            # ACT tick of the reduce for global tile T = b*NSUB + c*CHUNK + j
            b, r = divmod(T, NSUB)
            c, j = divmod(r, CHUNK)
            return ACT[red_ev(b, c, j)]

        # ---------- SYNC: all DMAs ----------
        @blk.sync
        def _(sync):
            sync.dma_start(
                out=wkq_sb[:], in_=wkq_d[:].rearrange("(dc p) a -> p dc a", p=128)
            ).then_inc(sW, 16)
            sync.dma_start(
                out=wkqT_sb[:], in_=wkqT_d[:].rearrange("(ac p) d -> p ac d", p=128)
            ).then_inc(sW, 16)
            with nc.allow_non_contiguous_dma(reason="tiny columnar q load"):
                sync.dma_start(
                    out=q_col[:], in_=q_d[:].rearrange("b (c p) -> p (b c)", p=128)
                ).then_inc(sW, 16)
            sync.dma_start(out=bkq_row[:], in_=bkq_d[:][None, :]).then_inc(sW, 16)
            sync.dma_start(out=bv_row[:], in_=bv_d[:][None, :]).then_inc(sBV, 16)

            for b in range(BPC):
                k_b = k_d[:][b].rearrange("(p s) d -> p s d", p=128)
                for c in range(NCHUNK):
                    g = b * NCHUNK + c
                    if g >= KBUFS:
                        gp = g - KBUFS
                        bp, cp = divmod(gp, NCHUNK)
                        sync.wait_ge(sDVE, DVE[mult_ev(bp, cp, CHUNK - 1)])
                    sync.dma_start(
                        out=kt[g % KBUFS][:],
                        in_=k_b[:, c * CHUNK:(c + 1) * CHUNK, :],
                    ).then_inc(sK[g % KBUFS], 16)
                if b == 0:
                    sync.dma_start(
                        out=wv_sb[:],
                        in_=wv_d[:].rearrange("(dc p) e -> p dc e", p=128),
                    ).then_inc(sWV, 16)
                v_b = v_d[:][b].rearrange("(p s) d -> p s d", p=128)
                for c in range(NCHUNK):
                    g = b * NCHUNK + c
                    if g >= VBUFS:
                        gp = g - VBUFS
                        bp, cp = divmod(gp, NCHUNK)
                        sync.wait_ge(sPE, PE[f"W{bp}C{cp}"])
                    sync.dma_start(
                        out=vt[g % VBUFS][:],
                        in_=v_b[:, c * CHUNK:(c + 1) * CHUNK, :],
                    ).then_inc(sV[g % VBUFS], 16)

            for b in range(BPC):
                sync.wait_ge(sDVE, DVE[f"OSB{b}"])
                sync.dma_start(out=out_d[:][b:b + 1, :], in_=o_sb[b][:]).then_inc(
                    sOUT, 16)
            sync.wait_ge(sOUT, BPC * 16)

        # ---------- PE ----------
        @blk.tensor
        def _(tensor):
            tensor.wait_ge(sW, 64)
            for b in range(BPC):
                # qp row = q^T @ W_kq -> ps_small[0:1, 256:512]
                if b > 0:
                    # bank-0 safety: prior batch's bank-0 reads done
                    tensor.wait_ge(sDVE, DVE[f"QPSB{b - 1}"])
                for dc in range(8):
                    mm = tensor.matmul(
                        out=ps_small[0:1, 256:256 + A],
                        lhsT=q_col[:, b * 8 + dc:b * 8 + dc + 1],
                        rhs=wkq_sb[:, dc, :],
                        start=(dc == 0), stop=(dc == 7),
                    )
                mm.then_inc(sPE, 1)                      # QPROW{b}

                # fold qp row -> columns ps_small[:, 0:2]
                tensor.wait_ge(sDVE, DVE[f"QPRSB{b}"])
                for c2 in range(2):
                    mm = tensor.matmul(
                        out=ps_small[:, c2:c2 + 1],
                        lhsT=qpr_sb[b][0:1, c2 * 128:(c2 + 1) * 128],
                        rhs=ones_row[0:1, 0:1],
                        start=True, stop=True,
                    )
                mm.then_inc(sPE, 1)                      # QPF{b}

                # qt row = qp^T @ W_kqT -> ps_a[0:1, :]
                tensor.wait_ge(sDVE, DVE[f"QPSB{b}"])
                if b > 0:
                    tensor.wait_ge(sACT, ACT[f"QTSB{b - 1}"])
                for ac in range(2):
                    for nh in range(2):
                        mm = tensor.matmul(
                            out=ps_a[0:1, nh * 512:(nh + 1) * 512],
                            lhsT=qp_sb[b][:, ac:ac + 1],
                            rhs=wkqT_sb[:, ac, nh * 512:(nh + 1) * 512],
                            start=(ac == 0), stop=(ac == 1),
                        )
                mm.then_inc(sPE, 1)                      # QT{b}

                # qt broadcast -> ps_b
                tensor.wait_ge(sACT, ACT[f"QTSB{b}"])
                for nh in range(2):
                    mm = tensor.matmul(
                        out=ps_b[:, nh * 512:(nh + 1) * 512],
                        lhsT=ones_row[:],
                        rhs=qt_sb[b][0:1, nh * 512:(nh + 1) * 512],
                        start=True, stop=True,
                    )
                mm.then_inc(sPE, 1)                      # QTB{b}

            for b in range(BPC):
                # Z = sum_p(psums)
                tensor.wait_ge(sACT, ACT[f"EXP{b}"])
                tensor.matmul(
                    out=ps_small[0:1, 4:5], lhsT=psums[b][:], rhs=ones_col[:],
                    start=True, stop=True,
                ).then_inc(sPE, 1)                       # Z{b}

                # w = attn_unnorm @ v -> ps_w[0:1, :]
                for c in range(NCHUNK):
                    g = b * NCHUNK + c
                    tensor.wait_ge(sV[g % VBUFS], (g // VBUFS + 1) * 16)
                    for j in range(CHUNK):
                        t = c * CHUNK + j
                        for nh in range(2):
                            mm = tensor.matmul(
                                out=ps_w[0:1, nh * 512:(nh + 1) * 512],
                                lhsT=pmat[b][:, t:t + 1],
                                rhs=vt[g % VBUFS][:, j, nh * 512:(nh + 1) * 512],
                                start=(t == 0), stop=(t == NSUB - 1),
                            )
                    mm.then_inc(sPE, 1)                  # W{b}C{c}

                # fold w row -> columns ps_small[:, 8:16]
                tensor.wait_ge(sACT, ACT[f"WROW{b}"])
                for dc in range(8):
                    mm = tensor.matmul(
                        out=ps_small[:, 8 + dc:9 + dc],
                        lhsT=w_row[b][0:1, dc * 128:(dc + 1) * 128],
                        rhs=ones_bf[0:1, 0:1],
                        start=True, stop=True,
                    )
                mm.then_inc(sPE, 1)                      # FOLD{b}

                # out row = (w/Z)^T @ W_v -> ps_a[0:1, :]
                tensor.wait_ge(sDVE, DVE[f"WCOL{b}"])
                if b == 0:
                    tensor.wait_ge(sWV, 16)
                    tensor.wait_ge(sACT, ACT["QTSB1"])   # ps_a overwrite guard
                for dc in range(8):
                    for nh in range(2):
                        mm = tensor.matmul(
                            out=ps_a[0:1, nh * 512:(nh + 1) * 512],
                            lhsT=w_col[b][:, dc:dc + 1],
                            rhs=wv_sb[:, dc, nh * 512:(nh + 1) * 512],
                            start=(dc == 0), stop=(dc == 7),
                        )
                mm.then_inc(sPE, 1)                      # PROJ{b}

        # ---------- DVE ----------
        @blk.vector
        def _(vector):
            vector.memset(ones_row[:], 1.0).then_inc(sDVE, 1)
            vector.memset(ones_col[:], 1.0).then_inc(sDVE, 1)
            vector.memset(ones_bf[:], 1.0).then_inc(sDVE, 1)

            def small_chain(b, step):
                if step == 0:
                    if b == 0:
                        vector.wait_ge(sW, 64)
                    vector.wait_ge(sPE, PE[f"QPROW{b}"])
                    vector.tensor_add(qpr_sb[b][:], ps_small[0:1, 256:256 + A],
                                      bkq_row[:]).then_inc(sDVE, 1)   # QPRSB{b}
                elif step == 1:
                    vector.wait_ge(sPE, PE[f"QPF{b}"])
                    vector.tensor_copy(out=qp_sb[b][:], in_=ps_small[:, 0:2]) \
                        .then_inc(sDVE, 1)                            # QPSB{b}
                else:
                    vector.wait_ge(sPE, PE[f"QTB{b}"])
                    vector.tensor_copy(out=qtb_sb[b][:], in_=ps_b[:]) \
                        .then_inc(sDVE, 1)                            # QTBSB{b}

            def mult_chunk(b, c):
                g = b * NCHUNK + c
                vector.wait_ge(sK[g % KBUFS], (g // KBUFS + 1) * 16)
                if c == 0:
                    # self-wait: qtb_sb copy completion before reads
                    vector.wait_ge(sDVE, DVE[f"QTBSB{b}"])
                for j in range(CHUNK):
                    T = b * NSUB + c * CHUNK + j
                    if T >= NSCR:
                        vector.wait_ge(sACT, tile_red_tick(T - NSCR))
                    vector.tensor_mul(
                        scr[T % NSCR][:], kt[g % KBUFS][:, j, :], qtb_sb[b][:]
                    ).then_inc(sDVE, 1)                   # MUL{b}_{c}_{j}

            def tail(b, step):
                if step == 0:
                    vector.wait_ge(sPE, PE[f"Z{b}"])
                    vector.reciprocal(invz[b][:], ps_small[0:1, 4:5]) \
                        .then_inc(sDVE, 1)                            # INVZ{b}
                elif step == 1:
                    vector.wait_ge(sPE, PE[f"FOLD{b}"])
                    vector.tensor_copy(out=w_col[b][:], in_=ps_small[:, 8:16]) \
                        .then_inc(sDVE, 1)                            # WCOL{b}
                else:
                    vector.wait_ge(sPE, PE[f"PROJ{b}"])
                    if b == 0:
                        vector.wait_ge(sBV, 16)
                    vector.tensor_add(o_sb[b][:], ps_a[0:1, :], bv_row[:]) \
                        .then_inc(sDVE, 1)                            # OSB{b}

            small_chain(0, 0)
            small_chain(0, 1)
            small_chain(0, 2)
            mult_chunk(0, 0)
            small_chain(1, 0)
            mult_chunk(0, 1)
            small_chain(1, 1)
            mult_chunk(0, 2)
            small_chain(1, 2)
            mult_chunk(0, 3)
            tail(0, 0)          # INVZ0
            mult_chunk(1, 0)
            mult_chunk(1, 1)
            tail(0, 1)          # WCOL0
            tail(0, 2)          # OSB0
            mult_chunk(1, 2)
            mult_chunk(1, 3)
            tail(1, 0)
            tail(1, 1)
            tail(1, 2)

        # ---------- ACT (scalar) ----------
        @blk.scalar
        def _(scalar):
            def qtsb(b):
                scalar.wait_ge(sPE, PE[f"QT{b}"])
                scalar.mul(qt_sb[b][:], ps_a[0:1, :], 1.0 / 16.0) \
                    .then_inc(sACT, 1)                                # QTSB{b}

            def red_chunk(b, c):
                for j in range(CHUNK):
                    T = b * NSUB + c * CHUNK + j
                    t = c * CHUNK + j
                    scalar.wait_ge(sDVE, DVE[mult_ev(b, c, j)])
                    scalar.activation(
                        out=scr[T % NSCR][:], in_=scr[T % NSCR][:], func=AF.Copy,
                        accum_out=smat[b][:, t:t + 1],
                    ).then_inc(sACT, 1)                   # RED{b}_{c}_{j}

            def expb(b):
                # self-wait: smat accum writes complete before exp reads
                scalar.wait_ge(sACT, ACT[red_ev(b, NCHUNK - 1, CHUNK - 1)])
                scalar.activation(
                    out=pmat[b][:], in_=smat[b][:], func=AF.Exp,
                    accum_out=psums[b][:],
                ).then_inc(sACT, 1)                                   # EXP{b}

            def wrow(b):
                scalar.wait_ge(sPE, PE[f"W{b}C{NCHUNK - 1}"])
                scalar.wait_ge(sDVE, DVE[f"INVZ{b}"])
                scalar.activation(
                    out=w_row[b][:], in_=ps_w[0:1, :], func=AF.Copy,
                    bias=0.0, scale=invz[b][0:1, 0:1],
                ).then_inc(sACT, 1)                                   # WROW{b}

            qtsb(0)
            red_chunk(0, 0)
            qtsb(1)
            for c in range(1, NCHUNK):
                red_chunk(0, c)
            expb(0)
            red_chunk(1, 0)
            red_chunk(1, 1)
            wrow(0)
            red_chunk(1, 2)
            red_chunk(1, 3)
            expb(1)
            wrow(1)

    return nc


_NC_CACHE = None


def get_nc():
    global _NC_CACHE
    if _NC_CACHE is None:
        _NC_CACHE = _build_nc()
    return _NC_CACHE


def make_in_maps(q, k, v, W_kq, b_kq, W_v, b_v):
    """Shard full inputs over 8 cores: batch-parallel, weights replicated.
    k, v, W_v are cast to bfloat16 on the host (compute dtype of the
    streaming contractions)."""
    import ml_dtypes

    bf16 = ml_dtypes.bfloat16
    q = np.ascontiguousarray(np.asarray(q, dtype=np.float32).reshape(B, E))
    k = np.ascontiguousarray(np.asarray(k, dtype=np.float32).astype(bf16))
    v = np.ascontiguousarray(np.asarray(v, dtype=np.float32).astype(bf16))
    W_kq = np.ascontiguousarray(np.asarray(W_kq, dtype=np.float32))
    W_kqT = np.ascontiguousarray(W_kq.T)
    b_kq = np.ascontiguousarray(np.asarray(b_kq, dtype=np.float32))
    W_v = np.ascontiguousarray(np.asarray(W_v, dtype=np.float32).astype(bf16))
    b_v = np.ascontiguousarray(np.asarray(b_v, dtype=np.float32))
    in_maps = []
    for i in range(NCORES):
        lo, hi = i * BPC, (i + 1) * BPC
        in_maps.append({
            "q": q[lo:hi],
            "k": k[lo:hi],
            "v": v[lo:hi],
            "W_kq": W_kq,
            "W_kqT": W_kqT,
            "b_kq": b_kq,
            "W_v": W_v,
            "b_v": b_v,
        })
    return in_maps


def kernel(q, k, v, W_kq, b_kq, W_v, b_v):
    from concourse.bass_utils import run_bass_kernel_spmd

    nc = get_nc()
    in_maps = make_in_maps(q, k, v, W_kq, b_kq, W_v, b_v)
    res = run_bass_kernel_spmd(nc, in_maps, core_ids=list(range(NCORES)))
    out = np.concatenate([res.results[i]["out"] for i in range(NCORES)], axis=0)
    return np.ascontiguousarray(out.astype(np.float32))


# revision 28
# speedup vs baseline: 2.3453x; 1.2271x over previous
"""Distributed attention kernel for Trainium2 (8 NeuronCores, SPMD).

Problem: B=16 batches of single-query attention over NK=4096 keys,
EMBED=1024, ATTN=256, with a shared kq projection and a v projection.

Math restructuring (exact up to float reassociation):
  - scores = (q@W_kq + b_kq) @ (k@W_kq + b_kq)^T / 16
           = k @ qt + const            where qt = W_kq @ (W_kq^T q + b_kq) / 16
    (the constant offsets every score equally -> softmax invariant, dropped)
  - out = softmax(scores) @ (v@W_v + b_v)
        = (attn @ v) @ W_v + b_v       (attn sums to 1)
This removes the O(NK*E*E) v-projection and O(NK*E*A) k-projection
entirely; the kernel is HBM-bandwidth bound streaming k and v once.

Sharding: data-parallel over batch, 2 batches per core. k, v, W_v are
cast to bf16 on the host (compute dtype of the streaming contractions,
halves HBM traffic); W_kq is additionally passed pre-transposed.

Token layout is p-major ("(p s) d"): partition p holds NSUB consecutive
token rows, so each chunk DMA is one 16KB-contiguous run per partition
(128 descriptors, not 1024). k and v use the same permutation, and
softmax is globally permutation-invariant, so results are unchanged.

s = k.qt is split across two engines per 128-token tile:
  DVE : prod = k_tile * qt_bcast   (bf16 tensor_tensor, 2x mode)
  ACT : activation(Copy) with accum_out -> row-sum = scores column
Softmax uses unnormalized exp (scores ~ N(0,1), no overflow in fp32)
with 1/Z folded into the w_row evacuation.

Raw bass (not Tile): this toolchain's walrus build rejects >1 embedded
sync-wait per compute instruction, which Tile's scheduler emits; raw
bass uses standalone sequencer waits, with explicit semaphore ticks
precomputed in python (the *_seq tables below).

PSUM bank map (PE-W vs DVE/ACT-R hazards serialized via the sem chain):
  bank 0    : qp_row [0:1,256:512], qp_col [:,0:2], Z [0:1,4:5], fold [:,8:16]
  banks 1-2 : qt row / out row (disjoint lifetimes)
  banks 3-4 : qt broadcast [128, 1024]
  banks 5-6 : w accumulator row
"""

import contextlib

import numpy as np

try:
    import concourse.bass as bass  # noqa: F401
except ImportError:  # fallback if site path isn't preloaded
    import sys

    sys.path.insert(0, "/opt/trn_rl_repo")

B = 16
NCORES = 8
BPC = B // NCORES  # batches per core
NK = 4096
E = 1024
A = 256
NSUB = NK // 128   # 32 token subtiles of 128
CHUNK = 8          # subtiles per DMA chunk (2 MB in bf16)
NCHUNK = NSUB // CHUNK
KBUFS = 3
VBUFS = 3
NACT = 5           # tiles per chunk reduced on ACT (rest fused on DVE)
# scratch slot j is dedicated to tile position j of each chunk, so a
# chunk's multiplies only wait on the PREVIOUS chunk's reduces


def _build_nc():
    import concourse.bass as bass
    from concourse import mybir

    FP = mybir.dt.float32
    BF = mybir.dt.bfloat16
    AL = mybir.AluOpType
    AF = mybir.ActivationFunctionType

    nc = bass.Bass()
    q_d = nc.declare_dram_parameter("q", [BPC, E], FP, isOutput=False)
    k_d = nc.declare_dram_parameter("k", [BPC, NK, E], BF, isOutput=False)
    v_d = nc.declare_dram_parameter("v", [BPC, NK, E], BF, isOutput=False)
    wkq_d = nc.declare_dram_parameter("W_kq", [E, A], FP, isOutput=False)
    wkqT_d = nc.declare_dram_parameter("W_kqT", [A, E], FP, isOutput=False)
    bkq_d = nc.declare_dram_parameter("b_kq", [A], FP, isOutput=False)
    wv_d = nc.declare_dram_parameter("W_v", [E, E], BF, isOutput=False)
    bv_d = nc.declare_dram_parameter("b_v", [E], FP, isOutput=False)
    out_d = nc.declare_dram_parameter("out", [BPC, E], FP, isOutput=True)

    with contextlib.ExitStack() as st:
        def sb(name, shape, dt=FP):
            return st.enter_context(nc.sbuf_tensor(name, shape, dt))

        # ---- SBUF ----
        wkq_sb = sb("wkq_sb", [128, 8, A])
        wkqT_sb = sb("wkqT_sb", [128, 2, E])
        wv_sb = sb("wv_sb", [128, 8, E], BF)
        q_col = sb("q_col", [128, BPC * 8])
        bkq_row = sb("bkq_row", [1, A])
        bv_row = sb("bv_row", [1, E])
        ones_row = sb("ones_row", [1, 128])        # fp32 (qt bcast / qp fold)
        ones_col = sb("ones_col", [128, 1])        # fp32 (Z rhs)
        ones_bf = sb("ones_bf", [1, 128], BF)      # bf16 (w fold rhs)
        kt = [sb(f"kt{i}", [128, CHUNK, E], BF) for i in range(KBUFS)]
        vt = [sb(f"vt{i}", [128, CHUNK, E], BF) for i in range(VBUFS)]
        scr = [sb(f"scr{i}", [128, E], BF) for i in range(NACT)]
        junk = sb("junk", [128, E], BF)
        # per-batch smalls
        qpr_sb = [sb(f"qpr_sb{b}", [1, A]) for b in range(BPC)]
        qp_sb = [sb(f"qp_sb{b}", [128, 2]) for b in range(BPC)]
        qt_sb = [sb(f"qt_sb{b}", [1, E]) for b in range(BPC)]
        qtb_sb = [sb(f"qtb_sb{b}", [128, E], BF) for b in range(BPC)]
        smat = [sb(f"smat{b}", [128, NSUB]) for b in range(BPC)]
        pmat = [sb(f"pmat{b}", [128, NSUB], BF) for b in range(BPC)]
        psums = [sb(f"psums{b}", [128, 1]) for b in range(BPC)]
        invz = [sb(f"invz{b}", [1, 1]) for b in range(BPC)]
        w_row = [sb(f"w_row{b}", [1, E], BF) for b in range(BPC)]
        w_col = [sb(f"w_col{b}", [128, 8], BF) for b in range(BPC)]
        o_sb = [sb(f"o_sb{b}", [1, E]) for b in range(BPC)]

        # ---- PSUM (static bank map) ----
        ps_small = st.enter_context(nc.psum_tensor([128, 512], FP))   # bank 0
        ps_a = st.enter_context(nc.psum_tensor([128, 1024], FP))      # banks 1-2
        ps_b = st.enter_context(nc.psum_tensor([128, 1024], FP))      # banks 3-4
        ps_w = st.enter_context(nc.psum_tensor([128, 1024], FP))      # banks 5-6

        # ---- semaphores ----
        sW = st.enter_context(nc.semaphore("sW"))      # wkq+wkqT+q+bkq -> 64
        sWV = st.enter_context(nc.semaphore("sWV"))    # wv -> 16
        sBV = st.enter_context(nc.semaphore("sBV"))    # bv -> 16
        sK = [st.enter_context(nc.semaphore(f"sK{i}")) for i in range(KBUFS)]
        sV = [st.enter_context(nc.semaphore(f"sV{i}")) for i in range(VBUFS)]
        sOUT = st.enter_context(nc.semaphore("sOUT"))
        sPE = st.enter_context(nc.semaphore("sPE"))
        sDVE = st.enter_context(nc.semaphore("sDVE"))
        sACT = st.enter_context(nc.semaphore("sACT"))

        blk = st.enter_context(nc.Block())

        # ---------- event tick registry ----------
        def ticks(seq):
            return {ev: i + 1 for i, ev in enumerate(seq)}

        pe_seq = []
        for b in range(BPC):
            pe_seq += [f"QPROW{b}", f"QPF{b}", f"QT{b}", f"QTB{b}"]
        for b in range(BPC):
            pe_seq += [f"Z{b}"]
            pe_seq += [f"W{b}C{c}" for c in range(NCHUNK)]
            pe_seq += [f"FOLD{b}", f"PROJ{b}"]
        PE = ticks(pe_seq)

        def mult_ev(b, c, j):
            # DVE inc for tile j of chunk (b, c): mult (j < NACT) or fused stt
            return f"MUL{b}_{c}_{j}"

        def red_ev(b, c, j):
            return f"RED{b}_{c}_{j}"

        dve_seq = ["MS1", "MS2", "MS3", "QPRSB0", "QPSB0", "QTBSB0"]
        dve_seq += [mult_ev(0, 0, j) for j in range(CHUNK)]
        dve_seq += ["QPRSB1"]
        dve_seq += [mult_ev(0, 1, j) for j in range(CHUNK)]
        dve_seq += ["QPSB1"]
        dve_seq += [mult_ev(0, 2, j) for j in range(CHUNK)]
        dve_seq += ["QTBSB1"]
        dve_seq += [mult_ev(0, 3, j) for j in range(CHUNK)]
        dve_seq += ["INVZ0"]
        dve_seq += [mult_ev(1, 0, j) for j in range(CHUNK)]
        dve_seq += ["WCOL0"]
        dve_seq += [mult_ev(1, 1, j) for j in range(CHUNK)]
        dve_seq += ["OSB0"]
        dve_seq += [mult_ev(1, 2, j) for j in range(CHUNK)]
        dve_seq += [mult_ev(1, 3, j) for j in range(CHUNK)]
        dve_seq += ["INVZ1", "WCOL1", "OSB1"]
        DVE = ticks(dve_seq)

        act_seq = ["QTSB0"]
        act_seq += [red_ev(0, 0, j) for j in range(NACT)]
        act_seq += ["QTSB1"]
        for c in range(1, NCHUNK):
            act_seq += [red_ev(0, c, j) for j in range(NACT)]
        act_seq += ["EXP0", "WROW0"]
        for c in range(NCHUNK):
            act_seq += [red_ev(1, c, j) for j in range(NACT)]
        act_seq += ["EXP1", "WROW1"]
        ACT = ticks(act_seq)

        # ---------- SYNC: all DMAs ----------
        @blk.sync
        def _(sync):
            sync.dma_start(
                out=wkq_sb[:], in_=wkq_d[:].rearrange("(dc p) a -> p dc a", p=128)
            ).then_inc(sW, 16)
            sync.dma_start(
                out=wkqT_sb[:], in_=wkqT_d[:].rearrange("(ac p) d -> p ac d", p=128)
            ).then_inc(sW, 16)
            with nc.allow_non_contiguous_dma(reason="tiny columnar q load"):
                sync.dma_start(
                    out=q_col[:], in_=q_d[:].rearrange("b (c p) -> p (b c)", p=128)
                ).then_inc(sW, 16)
            sync.dma_start(out=bkq_row[:], in_=bkq_d[:][None, :]).then_inc(sW, 16)
            sync.dma_start(out=bv_row[:], in_=bv_d[:][None, :]).then_inc(sBV, 16)

            for b in range(BPC):
                k_b = k_d[:][b].rearrange("(p s) d -> p s d", p=128)
                for c in range(NCHUNK):
                    g = b * NCHUNK + c
                    if g >= KBUFS:
                        gp = g - KBUFS
                        bp, cp = divmod(gp, NCHUNK)
                        sync.wait_ge(sDVE, DVE[mult_ev(bp, cp, CHUNK - 1)])
                    sync.dma_start(
                        out=kt[g % KBUFS][:],
                        in_=k_b[:, c * CHUNK:(c + 1) * CHUNK, :],
                    ).then_inc(sK[g % KBUFS], 16)
                if b == 0:
                    sync.dma_start(
                        out=wv_sb[:],
                        in_=wv_d[:].rearrange("(dc p) e -> p dc e", p=128),
                    ).then_inc(sWV, 16)
                v_b = v_d[:][b].rearrange("(p s) d -> p s d", p=128)
                for c in range(NCHUNK):
                    g = b * NCHUNK + c
                    if g >= VBUFS:
                        gp = g - VBUFS
                        bp, cp = divmod(gp, NCHUNK)
                        sync.wait_ge(sPE, PE[f"W{bp}C{cp}"])
                    sync.dma_start(
                        out=vt[g % VBUFS][:],
                        in_=v_b[:, c * CHUNK:(c + 1) * CHUNK, :],
                    ).then_inc(sV[g % VBUFS], 16)

            for b in range(BPC):
                sync.wait_ge(sDVE, DVE[f"OSB{b}"])
                sync.dma_start(out=out_d[:][b:b + 1, :], in_=o_sb[b][:]).then_inc(
                    sOUT, 16)
            sync.wait_ge(sOUT, BPC * 16)

        # ---------- PE ----------
        @blk.tensor
        def _(tensor):
            tensor.wait_ge(sW, 64)
            for b in range(BPC):
                # qp row = q^T @ W_kq -> ps_small[0:1, 256:512]
                if b > 0:
                    # bank-0 safety: prior batch's bank-0 reads done
                    tensor.wait_ge(sDVE, DVE[f"QPSB{b - 1}"])
                for dc in range(8):
                    mm = tensor.matmul(
                        out=ps_small[0:1, 256:256 + A],
                        lhsT=q_col[:, b * 8 + dc:b * 8 + dc + 1],
                        rhs=wkq_sb[:, dc, :],
                        start=(dc == 0), stop=(dc == 7),
                    )
                mm.then_inc(sPE, 1)                      # QPROW{b}

                # fold qp row -> columns ps_small[:, 0:2]
                tensor.wait_ge(sDVE, DVE[f"QPRSB{b}"])
                for c2 in range(2):
                    mm = tensor.matmul(
                        out=ps_small[:, c2:c2 + 1],
                        lhsT=qpr_sb[b][0:1, c2 * 128:(c2 + 1) * 128],
                        rhs=ones_row[0:1, 0:1],
                        start=True, stop=True,
                    )
                mm.then_inc(sPE, 1)                      # QPF{b}

                # qt row = qp^T @ W_kqT -> ps_a[0:1, :]
                tensor.wait_ge(sDVE, DVE[f"QPSB{b}"])
                if b > 0:
                    tensor.wait_ge(sACT, ACT[f"QTSB{b - 1}"])
                for ac in range(2):
                    for nh in range(2):
                        mm = tensor.matmul(
                            out=ps_a[0:1, nh * 512:(nh + 1) * 512],
                            lhsT=qp_sb[b][:, ac:ac + 1],
                            rhs=wkqT_sb[:, ac, nh * 512:(nh + 1) * 512],
                            start=(ac == 0), stop=(ac == 1),
                        )
                mm.then_inc(sPE, 1)                      # QT{b}

                # qt broadcast -> ps_b
                tensor.wait_ge(sACT, ACT[f"QTSB{b}"])
                for nh in range(2):
                    mm = tensor.matmul(
                        out=ps_b[:, nh * 512:(nh + 1) * 512],
                        lhsT=ones_row[:],
                        rhs=qt_sb[b][0:1, nh * 512:(nh + 1) * 512],
                        start=True, stop=True,
                    )
                mm.then_inc(sPE, 1)                      # QTB{b}

            for b in range(BPC):
                # Z = sum_p(psums)
                tensor.wait_ge(sACT, ACT[f"EXP{b}"])
                tensor.matmul(
                    out=ps_small[0:1, 4:5], lhsT=psums[b][:], rhs=ones_col[:],
                    start=True, stop=True,
                ).then_inc(sPE, 1)                       # Z{b}

                # w = attn_unnorm @ v -> ps_w[0:1, :]
                for c in range(NCHUNK):
                    g = b * NCHUNK + c
                    tensor.wait_ge(sV[g % VBUFS], (g // VBUFS + 1) * 16)
                    for j in range(CHUNK):
                        t = c * CHUNK + j
                        for nh in range(2):
                            mm = tensor.matmul(
                                out=ps_w[0:1, nh * 512:(nh + 1) * 512],
                                lhsT=pmat[b][:, t:t + 1],
                                rhs=vt[g % VBUFS][:, j, nh * 512:(nh + 1) * 512],
                                start=(t == 0), stop=(t == NSUB - 1),
                            )
                    mm.then_inc(sPE, 1)                  # W{b}C{c}

                # fold w row -> columns ps_small[:, 8:16]
                tensor.wait_ge(sACT, ACT[f"WROW{b}"])
                for dc in range(8):
                    mm = tensor.matmul(
                        out=ps_small[:, 8 + dc:9 + dc],
                        lhsT=w_row[b][0:1, dc * 128:(dc + 1) * 128],
                        rhs=ones_bf[0:1, 0:1],
                        start=True, stop=True,
                    )
                mm.then_inc(sPE, 1)                      # FOLD{b}

                # out row = (w/Z)^T @ W_v -> ps_a[0:1, :]
                tensor.wait_ge(sDVE, DVE[f"WCOL{b}"])
                if b == 0:
                    tensor.wait_ge(sWV, 16)
                    tensor.wait_ge(sACT, ACT["QTSB1"])   # ps_a overwrite guard
                for dc in range(8):
                    for nh in range(2):
                        mm = tensor.matmul(
                            out=ps_a[0:1, nh * 512:(nh + 1) * 512],
                            lhsT=w_col[b][:, dc:dc + 1],
                            rhs=wv_sb[:, dc, nh * 512:(nh + 1) * 512],
                            start=(dc == 0), stop=(dc == 7),
                        )
                mm.then_inc(sPE, 1)                      # PROJ{b}

        # ---------- DVE ----------
        @blk.vector
        def _(vector):
            vector.memset(ones_row[:], 1.0).then_inc(sDVE, 1)
            vector.memset(ones_col[:], 1.0).then_inc(sDVE, 1)
            vector.memset(ones_bf[:], 1.0).then_inc(sDVE, 1)

            def small_chain(b, step):
                if step == 0:
                    if b == 0:
                        vector.wait_ge(sW, 64)
                    vector.wait_ge(sPE, PE[f"QPROW{b}"])
                    vector.tensor_add(qpr_sb[b][:], ps_small[0:1, 256:256 + A],
                                      bkq_row[:]).then_inc(sDVE, 1)   # QPRSB{b}
                elif step == 1:
                    vector.wait_ge(sPE, PE[f"QPF{b}"])
                    vector.tensor_copy(out=qp_sb[b][:], in_=ps_small[:, 0:2]) \
                        .then_inc(sDVE, 1)                            # QPSB{b}
                else:
                    vector.wait_ge(sPE, PE[f"QTB{b}"])
                    vector.tensor_copy(out=qtb_sb[b][:], in_=ps_b[:]) \
                        .then_inc(sDVE, 1)                            # QTBSB{b}

            def mult_chunk(b, c):
                g = b * NCHUNK + c
                vector.wait_ge(sK[g % KBUFS], (g // KBUFS + 1) * 16)
                if c == 0:
                    # self-wait: qtb_sb copy completion before reads
                    vector.wait_ge(sDVE, DVE[f"QTBSB{b}"])
                for j in range(CHUNK):
                    t = c * CHUNK + j
                    if j < NACT:
                        if g >= 1:
                            # scratch slot j: previous chunk's reduce done
                            bp, cp = divmod(g - 1, NCHUNK)
                            vector.wait_ge(sACT, ACT[red_ev(bp, cp, j)])
                        vector.tensor_mul(
                            scr[j][:], kt[g % KBUFS][:, j, :], qtb_sb[b][:]
                        ).then_inc(sDVE, 1)               # MUL{b}_{c}_{j}
                    else:
                        if g >= 1:
                            # junk WAW: self-wait (always satisfied in-order;
                            # appeases the address-level race detector)
                            bp, cp = divmod(g - 1, NCHUNK)
                            vector.wait_ge(sDVE, DVE[mult_ev(bp, cp, j)])
                        # fused dot product on DVE: out=(k*1)*qt, accum=row sum
                        vector.scalar_tensor_tensor(
                            out=junk[:], in0=kt[g % KBUFS][:, j, :], scalar=1.0,
                            in1=qtb_sb[b][:],
                            op0=AL.mult, op1=AL.mult,
                            accum_out=smat[b][:, t:t + 1],
                        ).then_inc(sDVE, 1)               # MUL{b}_{c}_{j}

            def tail(b, step):
                if step == 0:
                    vector.wait_ge(sPE, PE[f"Z{b}"])
                    vector.reciprocal(invz[b][:], ps_small[0:1, 4:5]) \
                        .then_inc(sDVE, 1)                            # INVZ{b}
                elif step == 1:
                    vector.wait_ge(sPE, PE[f"FOLD{b}"])
                    vector.tensor_copy(out=w_col[b][:], in_=ps_small[:, 8:16]) \
                        .then_inc(sDVE, 1)                            # WCOL{b}
                else:
                    vector.wait_ge(sPE, PE[f"PROJ{b}"])
                    if b == 0:
                        vector.wait_ge(sBV, 16)
                    vector.tensor_add(o_sb[b][:], ps_a[0:1, :], bv_row[:]) \
                        .then_inc(sDVE, 1)                            # OSB{b}

            small_chain(0, 0)
            small_chain(0, 1)
            small_chain(0, 2)
            mult_chunk(0, 0)
            small_chain(1, 0)
            mult_chunk(0, 1)
            small_chain(1, 1)
            mult_chunk(0, 2)
            small_chain(1, 2)
            mult_chunk(0, 3)
            tail(0, 0)          # INVZ0
            mult_chunk(1, 0)
            tail(0, 1)          # WCOL0
            mult_chunk(1, 1)
            tail(0, 2)          # OSB0
            mult_chunk(1, 2)
            mult_chunk(1, 3)
            tail(1, 0)
            tail(1, 1)
            tail(1, 2)

        # ---------- ACT (scalar) ----------
        @blk.scalar
        def _(scalar):
            def qtsb(b):
                scalar.wait_ge(sPE, PE[f"QT{b}"])
                scalar.mul(qt_sb[b][:], ps_a[0:1, :], 1.0 / 16.0) \
                    .then_inc(sACT, 1)                                # QTSB{b}

            def red_chunk(b, c):
                for j in range(NACT):
                    t = c * CHUNK + j
                    scalar.wait_ge(sDVE, DVE[mult_ev(b, c, j)])
                    scalar.activation(
                        out=scr[j][:], in_=scr[j][:], func=AF.Copy,
                        accum_out=smat[b][:, t:t + 1],
                    ).then_inc(sACT, 1)                   # RED{b}_{c}_{j}

            def expb(b):
                # smat writers: ACT reduces (self-order) + DVE fused stts
                scalar.wait_ge(sACT, ACT[red_ev(b, NCHUNK - 1, NACT - 1)])
                scalar.wait_ge(sDVE, DVE[mult_ev(b, NCHUNK - 1, CHUNK - 1)])
                scalar.activation(
                    out=pmat[b][:], in_=smat[b][:], func=AF.Exp,
                    accum_out=psums[b][:],
                ).then_inc(sACT, 1)                                   # EXP{b}

            def wrow(b):
                scalar.wait_ge(sPE, PE[f"W{b}C{NCHUNK - 1}"])
                scalar.wait_ge(sDVE, DVE[f"INVZ{b}"])
                scalar.activation(
                    out=w_row[b][:], in_=ps_w[0:1, :], func=AF.Copy,
                    bias=0.0, scale=invz[b][0:1, 0:1],
                ).then_inc(sACT, 1)                                   # WROW{b}

            qtsb(0)
            red_chunk(0, 0)
            qtsb(1)
            for c in range(1, NCHUNK):
                red_chunk(0, c)
            expb(0)
            wrow(0)
            for c in range(NCHUNK):
                red_chunk(1, c)
            expb(1)
            wrow(1)

    return nc


_NC_CACHE = None


def get_nc():
    global _NC_CACHE
    if _NC_CACHE is None:
        _NC_CACHE = _build_nc()
    return _NC_CACHE


def make_in_maps(q, k, v, W_kq, b_kq, W_v, b_v):
    """Shard full inputs over 8 cores: batch-parallel, weights replicated.
    k, v, W_v are cast to bfloat16 on the host (compute dtype of the
    streaming contractions)."""
    import ml_dtypes

    bf16 = ml_dtypes.bfloat16
    q = np.ascontiguousarray(np.asarray(q, dtype=np.float32).reshape(B, E))
    k = np.ascontiguousarray(np.asarray(k, dtype=np.float32).astype(bf16))
    v = np.ascontiguousarray(np.asarray(v, dtype=np.float32).astype(bf16))
    W_kq = np.ascontiguousarray(np.asarray(W_kq, dtype=np.float32))
    W_kqT = np.ascontiguousarray(W_kq.T)
    b_kq = np.ascontiguousarray(np.asarray(b_kq, dtype=np.float32))
    W_v = np.ascontiguousarray(np.asarray(W_v, dtype=np.float32).astype(bf16))
    b_v = np.ascontiguousarray(np.asarray(b_v, dtype=np.float32))
    in_maps = []
    for i in range(NCORES):
        lo, hi = i * BPC, (i + 1) * BPC
        in_maps.append({
            "q": q[lo:hi],
            "k": k[lo:hi],
            "v": v[lo:hi],
            "W_kq": W_kq,
            "W_kqT": W_kqT,
            "b_kq": b_kq,
            "W_v": W_v,
            "b_v": b_v,
        })
    return in_maps


def kernel(q, k, v, W_kq, b_kq, W_v, b_v):
    from concourse.bass_utils import run_bass_kernel_spmd

    nc = get_nc()
    in_maps = make_in_maps(q, k, v, W_kq, b_kq, W_v, b_v)
    res = run_bass_kernel_spmd(nc, in_maps, core_ids=list(range(NCORES)))
    out = np.concatenate([res.results[i]["out"] for i in range(NCORES)], axis=0)
    return np.ascontiguousarray(out.astype(np.float32))
